# revision 25
# baseline (speedup 1.0000x reference)
"""v5: fp32r matmuls replace bf16 hi/lo (PE passes 13 -> 7 per step-column);
spk2 output dropped (host thresholds the exact fp32 mem2); slab-merged 3D
tiles halve vector-op count; whole-step DMAs (one in, one out per step).

Per step t (threshold 1.0, decay beta), spikes s1 in {-1,+1}, s2 in {0,1}:
  ps1 = W1r @ [x_t; ones]  + R1(-I/2) @ s1_{t-1}       [PE, f32r]
        (W1r ones-row = -1/2 completes -(s1+1)/2 reset)
  m1  = beta*m1 + ps1                                  [DVE, fp32 exact]
  s1  = Sign(m1 - 1)  in {-1,+1} f32r                  [ACT]
  ps2 = W2a@[s1;1](slab0) + W2b@[s1;1](slab1) + R2(-I)@s2   [PE, f32r]
        (W2' = w2/2, bias row = sum(w2)/2; R2=-I acts on {0,1} s2)
  m2  = beta*m2 + ps2    -> DRAM (fp32, per-step, double-buffered)
  s2  = (m2 > 1) in {0,1} f32r                         [Pool]

f32r rounds matmul operands to ~12-13 mantissa bits; emulated end-to-end
error vs the fp32 reference is ~6e-3 (gate 2e-2). The fp32 state path
(DVE) and the DRAM mem2 are exact fp32, so host-side spk2 = (mem2 > 1)
reproduces the device threshold decision bit-exactly.

Lane layout per core: rows r = 4*bl+i (x) / 5*bl+h (m1/s1) for 21 batch
lanes; slab dim s in {0,1} is the middle axis of 3D tiles; l2 pair-packs
both slabs into 126 = 2*63 psum rows (r = 63*s + 3*bl + o).
"""

import numpy as np
from contextlib import ExitStack
from concurrent.futures import ThreadPoolExecutor

T = 10
NI, NH, NO = 4, 5, 3
BETA = 0.95
THR = 1.0
B_FULL = 1_000_000
NCORES = 8

NBL = 21
NSLAB = 2
NCOLS = 2978
NPB = 512
BC = NSLAB * NBL * NCOLS  # 125,076
BPAD = BC * NCORES        # 1,000,608

XR = NBL * NI             # 84
M1 = NBL * NH             # 105
M2 = NBL * NO             # 63
M2P = 2 * M2              # 126

V5_OPTS = dict(loop_split=True, l2w=1024, m2_path="dve")


def make_weights(w1, w2):
    w1 = np.asarray(w1, np.float32)
    w2 = np.asarray(w2, np.float32)
    # W1 [85, 128]: rows (bl,i) -> w1[h,i] at col 5bl+h; ones row -> -1/2
    w1f = np.zeros((XR + 1, 128), np.float32)
    for bl in range(NBL):
        for i in range(NI):
            for h in range(NH):
                w1f[4 * bl + i, 5 * bl + h] = w1[h, i]
    w1f[XR, 0:M1] = -0.5
    # R1 [105, 128]: -I/2 on +-1 spikes
    r1 = np.zeros((M1, 128), np.float32)
    r1[:, 0:M1] = -0.5 * np.eye(M1, dtype=np.float32)
    # W2' per slab [106, 126]: w2/2 at [5bl+h, 63s+3bl+o]; bias row sum(w2)/2
    w2f = np.zeros((2, M1 + 1, M2P), np.float32)
    for s in range(2):
        for bl in range(NBL):
            for h in range(NH):
                for o in range(NO):
                    w2f[s, 5 * bl + h, 63 * s + 3 * bl + o] = w2[o, h] / 2.0
        for bl in range(NBL):
            for o in range(NO):
                w2f[s, M1, 63 * s + 3 * bl + o] = float(
                    w2[o].astype(np.float64).sum()) / 2.0
    # R2 [126, 126]: -I on {0,1} spikes
    r2 = (-np.eye(M2P)).astype(np.float32)
    # beta*I for the PE-side m2 decay (m2_path != 'dve')
    bI = (BETA * np.eye(M2P)).astype(np.float32)
    return w1f, r1, w2f, r2, bI


def _split_multi_waits(nc):
    """Walrus codegen for compute-engine ISA slots accepts only ONE sync-wait
    command per instruction. Tile sometimes attaches 2+ (e.g. own-engine sem +
    a DMA-completion lane). Hoist the extras onto pure-sync EventSemaphore
    instructions inserted just before, on the same engine queue."""
    import concourse.mybir as mybir

    for f in nc.m.functions:
        for blk in f.blocks:
            out = []
            for ins in blk.instructions:
                si = ins.sync_info
                if (
                    si is not None
                    and len(si.on_wait) > 1
                    and not isinstance(ins, mybir.InstEventSemaphore)
                ):
                    waits = list(si.on_wait)
                    for j, w in enumerate(waits[:-1]):
                        out.append(
                            mybir.InstEventSemaphore(
                                name=f"{ins.name}-ws{j}",
                                engine=ins.engine,
                                ins=[],
                                outs=[],
                                sync_info=mybir.SyncInfo(
                                    on_wait=[w], on_update=[]
                                ),
                            )
                        )
                    ins.sync_info = mybir.SyncInfo(
                        on_wait=[waits[-1]], on_update=list(si.on_update)
                    )
                out.append(ins)
            blk.instructions = out


def build_nc_v5(split_waits=True, loop_split=True, l2w=1024, m2_path="dve"):
    """m2_path: 'dve' = exact fp32 stt on DVE; 'act'/'pool' = beta*I matmul
    into psum + Copy on ACT/Pool (m2 state then lives in f32r)."""
    import concourse.bass as bass
    import concourse.mybir as mybir
    from concourse.tile import TileContext

    f32 = mybir.dt.float32
    f32r = mybir.dt.float32r
    Act = mybir.ActivationFunctionType
    AOp = mybir.AluOpType

    def mkgroups(w):
        out, c0 = [], 0
        while c0 < NCOLS:
            n = min(w, NCOLS - c0)
            out.append((c0, n))
            c0 += n
        return out

    groups = mkgroups(NPB)
    groups2 = mkgroups(l2w) if loop_split else groups
    m2_f32r = m2_path != "dve"
    m2_dt_np = np.float32

    nc = bass.Bass()
    x_d = nc.declare_dram_parameter("x", [T, XR, NSLAB, NCOLS], f32r,
                                    isOutput=False)
    w1_d = nc.declare_dram_parameter("w1", [XR + 1, 128], f32r, isOutput=False)
    r1_d = nc.declare_dram_parameter("r1", [M1, 128], f32r, isOutput=False)
    w2a_d = nc.declare_dram_parameter("w2a", [M1 + 1, M2P], f32r, isOutput=False)
    w2b_d = nc.declare_dram_parameter("w2b", [M1 + 1, M2P], f32r, isOutput=False)
    r2_d = nc.declare_dram_parameter("r2", [M2P, M2P], f32r, isOutput=False)
    if m2_f32r:
        bI_d = nc.declare_dram_parameter("bI", [M2P, M2P], f32r,
                                         isOutput=False)
    ones_d = nc.declare_dram_parameter("ones", [1, NSLAB * NCOLS], f32r,
                                       isOutput=False)
    m2_dt = f32r if m2_f32r else f32
    mem_d = nc.declare_dram_parameter("mem2", [T, M2P, NCOLS], m2_dt,
                                      isOutput=True)

    with ExitStack() as ctx:
        tc = ctx.enter_context(TileContext(nc))
        wp = ctx.enter_context(tc.tile_pool(name="wp", bufs=1))
        st = ctx.enter_context(tc.tile_pool(name="st", bufs=1))
        xp = ctx.enter_context(tc.tile_pool(name="xp", bufs=1))
        psa = ctx.enter_context(tc.tile_pool(
            name="psa", bufs=2 if loop_split else 3, space="PSUM"))
        psb = ctx.enter_context(tc.tile_pool(name="psb", bufs=2, space="PSUM"))

        negone = wp.tile([128, 1], f32, tag="negone")
        nc.vector.memset(negone[:], -1.0)
        w1 = wp.tile([XR + 1, 128], f32r, tag="w1")
        r1 = wp.tile([M1, 128], f32r, tag="r1")
        w2a = wp.tile([M1 + 1, M2P], f32r, tag="w2a")
        w2b = wp.tile([M1 + 1, M2P], f32r, tag="w2b")
        r2 = wp.tile([M2P, M2P], f32r, tag="r2")
        wdmas = [(w1, w1_d), (r1, r1_d), (w2a, w2a_d), (w2b, w2b_d),
                 (r2, r2_d)]
        if m2_f32r:
            bI = wp.tile([M2P, M2P], f32r, tag="bI")
            wdmas.append((bI, bI_d))
        for tl, dr in wdmas:
            nc.sync.dma_start(tl[:], dr[:])

        # persistent state; no zero-init needed: step 0 skips the reset
        # matmuls (resets are zero) and copies psum instead of stt.
        m1t = st.tile([M1, NSLAB, NCOLS], f32, tag="m1", name="m1")
        s1t = st.tile([M1 + 1, NSLAB, NCOLS], f32r, tag="s1", name="s1")
        s2t = st.tile([M2P, NCOLS], f32r, tag="s2", name="s2")
        m2t = [st.tile([M2P, NCOLS], m2_dt, tag=f"m2_{k}", name=f"m2_{k}")
               for k in range(2)]
        # x ring: 3 step-tiles; row 84 = ones (set once via DMA)
        xs = [xp.tile([XR + 1, NSLAB, NCOLS], f32r, tag=f"x_{r}",
                      name=f"x_{r}") for r in range(3)]

        nc.sync.dma_start(s1t[M1:M1 + 1, :, :], ones_d[:])
        for r in range(3):
            nc.sync.dma_start(xs[r][XR:XR + 1, :, :], ones_d[:])

        def xdma(t):
            nc.sync.dma_start(xs[t % 3][0:XR, :, :], x_d[t])

        xdma(0)
        xdma(1)

        def l1(t, gi, c0, n):
            xt = xs[t % 3]
            cs = slice(c0, c0 + n)
            ps1 = psa.tile([128, NSLAB, NPB], f32, tag="ps1",
                           name=f"ps1_{t}_{gi}")
            if t == 0:
                # reset1 = 0: no ones row (-1/2) and no r1 matmul
                for s in range(NSLAB):
                    nc.tensor.matmul(ps1[:, s, 0:n], w1[0:XR, :],
                                     xt[0:XR, s, cs], start=True, stop=True)
                nc.vector.tensor_copy(m1t[:, :, cs], ps1[0:M1, :, 0:n])
            else:
                for s in range(NSLAB):
                    nc.tensor.matmul(ps1[:, s, 0:n], w1[:], xt[:, s, cs],
                                     start=True, stop=False)
                    nc.tensor.matmul(ps1[:, s, 0:n], r1[:],
                                     s1t[0:M1, s, cs],
                                     start=False, stop=True)
                # m1 = beta*m1 + ps1  (both slabs in one op)
                nc.vector.scalar_tensor_tensor(
                    m1t[:, :, cs], m1t[:, :, cs], BETA,
                    ps1[0:M1, :, 0:n], AOp.mult, AOp.add)
            # s1 = Sign(m1 - 1) in f32r
            nc.scalar.activation(
                s1t[0:M1, :, cs], m1t[:, :, cs], Act.Sign,
                bias=negone[0:M1, :])

        def l2(t, gi, c0, n, mo, mp):
            cs = slice(c0, c0 + n)
            ps2 = psb.tile([M2P, l2w], f32, tag="ps2", name=f"ps2_{t}_{gi}")

            def mm(w, rhs_fn, start, stop):
                o = 0
                while o < n:
                    k = min(512, n - o)
                    nc.tensor.matmul(ps2[:, o:o + k], w,
                                     rhs_fn(c0 + o, c0 + o + k),
                                     start=start, stop=stop)
                    o += k

            mm(w2a[:], lambda a, b: s1t[:, 0, a:b], True, False)
            mm(w2b[:], lambda a, b: s1t[:, 1, a:b], False, t == 0)
            if t > 0:
                mm(r2[:], lambda a, b: s2t[:, a:b], False, not m2_f32r)
                if m2_f32r:
                    mm(bI[:], lambda a, b: mo[:, a:b], False, True)
            # m2 = beta*m2_prev + ps2  (ping-pong state for lazy DMA)
            if m2_path == "dve":
                if t == 0:
                    nc.vector.tensor_copy(mp[:, cs], ps2[:, 0:n])
                else:
                    nc.vector.scalar_tensor_tensor(
                        mp[:, cs], mo[:, cs], BETA, ps2[:, 0:n],
                        AOp.mult, AOp.add)
            elif m2_path == "act":
                nc.scalar.activation(mp[:, cs], ps2[:, 0:n], Act.Copy)
            else:
                nc.gpsimd.tensor_copy(mp[:, cs], ps2[:, 0:n])
            # s2 = (m2 > 1) in {0,1} f32r
            nc.gpsimd.tensor_scalar(s2t[:, cs], mp[:, cs], THR, None,
                                    AOp.is_gt)

        for t in range(T):
            if t + 2 < T:
                xdma(t + 2)
            mo, mp = m2t[t % 2], m2t[(t + 1) % 2]
            if loop_split:
                for gi, (c0, n) in enumerate(groups):
                    l1(t, gi, c0, n)
                for gi, (c0, n) in enumerate(groups2):
                    l2(t, gi, c0, n, mo, mp)
            else:
                for gi, (c0, n) in enumerate(groups):
                    l1(t, gi, c0, n)
                    l2(t, gi, c0, n, mo, mp)
            nc.sync.dma_start(mem_d[t], mp[:])

    if split_waits:
        _split_multi_waits(nc)
    return nc


def make_weights_v6(w1, w2):
    """Layer-2 state is tracked as v(t) = mem2(t)/beta^t, accumulated in
    PSUM across all T steps (PE start=False). The decay folds into
    t-scaled stationaries: W2'_t = (w2/2)*beta^-t (+bias row), r2_t =
    -beta^-t * I. Spike threshold for v is thr_t = beta^-t."""
    w1f, r1, w2f, r2_, bI_ = make_weights(w1, w2)
    sc = (1.0 / np.float64(BETA) ** np.arange(T)).astype(np.float64)
    w2t = np.zeros((2, T, M1 + 1, M2P), np.float32)
    r2t = np.zeros((T, M2P, M2P), np.float32)
    for t in range(T):
        w2t[0, t] = (w2f[0].astype(np.float64) * sc[t]).astype(np.float32)
        w2t[1, t] = (w2f[1].astype(np.float64) * sc[t]).astype(np.float32)
        r2t[t] = (-sc[t] * np.eye(M2P)).astype(np.float32)
    thr = sc.astype(np.float32)  # thr_t = beta^-t * THR (THR=1)
    return w1f, r1, w2t, r2t, thr


def build_nc_v6(split_waits=True):
    import concourse.bass as bass
    import concourse.mybir as mybir
    from concourse.tile import TileContext

    f32 = mybir.dt.float32
    f32r = mybir.dt.float32r
    bf16 = mybir.dt.bfloat16
    Act = mybir.ActivationFunctionType
    AOp = mybir.AluOpType

    groups = []
    c0 = 0
    while c0 < NCOLS:
        n = min(NPB, NCOLS - c0)
        groups.append((c0, n))
        c0 += n
    NG = len(groups)

    # host-exact beta^-t thresholds (float32)
    thr_t = [np.float32(1.0 / np.float64(BETA) ** t) for t in range(T)]

    nc = bass.Bass()
    x_d = nc.declare_dram_parameter("x", [T, XR, NSLAB, NCOLS], f32r,
                                    isOutput=False)
    w1_d = nc.declare_dram_parameter("w1", [XR + 1, 128], f32r, isOutput=False)
    r1_d = nc.declare_dram_parameter("r1", [M1, 128], f32r, isOutput=False)
    w2a_d = nc.declare_dram_parameter("w2a", [M1 + 1, T, M2P], f32r,
                                      isOutput=False)
    w2b_d = nc.declare_dram_parameter("w2b", [M1 + 1, T, M2P], f32r,
                                      isOutput=False)
    r2_d = nc.declare_dram_parameter("r2", [M2P, T, M2P], f32r,
                                     isOutput=False)
    ones_d = nc.declare_dram_parameter("ones", [1, NSLAB * NCOLS], f32r,
                                       isOutput=False)
    out_d = nc.declare_dram_parameter("vout", [T, M2P, NCOLS], bf16,
                                      isOutput=True)

    with ExitStack() as ctx:
        tc = ctx.enter_context(TileContext(nc))
        wp = ctx.enter_context(tc.tile_pool(name="wp", bufs=1))
        st = ctx.enter_context(tc.tile_pool(name="st", bufs=1))
        xp = ctx.enter_context(tc.tile_pool(name="xp", bufs=1))
        psa = ctx.enter_context(tc.tile_pool(name="psa", bufs=1, space="PSUM"))
        psv = ctx.enter_context(tc.tile_pool(name="psv", bufs=1, space="PSUM"))

        negone = wp.tile([128, 1], f32, tag="negone")
        nc.vector.memset(negone[:], -1.0)
        w1 = wp.tile([XR + 1, 128], f32r, tag="w1")
        r1 = wp.tile([M1, 128], f32r, tag="r1")
        w2a = wp.tile([M1 + 1, T, M2P], f32r, tag="w2a")
        w2b = wp.tile([M1 + 1, T, M2P], f32r, tag="w2b")
        r2 = wp.tile([M2P, T, M2P], f32r, tag="r2")
        for tl, dr in ((w1, w1_d), (r1, r1_d), (w2a, w2a_d), (w2b, w2b_d),
                       (r2, r2_d)):
            nc.sync.dma_start(tl[:], dr[:])

        m1t = st.tile([M1, NSLAB, NCOLS], f32, tag="m1", name="m1")
        s1t = st.tile([M1 + 1, NSLAB, NCOLS], f32r, tag="s1", name="s1")
        s2t = st.tile([M2P, NCOLS], f32r, tag="s2", name="s2")
        # bf16 staging of (v - thr_t): spike sign exact under rounding
        vo = [st.tile([M2P, NCOLS], bf16, tag=f"vo_{k}", name=f"vo_{k}")
              for k in range(2)]
        # v = mem2/beta^t accumulated in PSUM across all steps (6 banks)
        v = psv.tile([M2P, NCOLS], f32, tag="v", name="v")
        xs = [xp.tile([XR + 1, NSLAB, NCOLS], f32r, tag=f"x_{r}",
                      name=f"x_{r}") for r in range(3)]

        nc.sync.dma_start(s1t[M1:M1 + 1, :, :], ones_d[:])
        for r in range(3):
            nc.sync.dma_start(xs[r][XR:XR + 1, :, :], ones_d[:])

        def xdma(t):
            nc.sync.dma_start(xs[t % 3][0:XR, :, :], x_d[t])

        xdma(0)
        xdma(1)

        def l1(t, gi, c0, n):
            xt = xs[t % 3]
            cs = slice(c0, c0 + n)
            ps1 = psa.tile([128, NSLAB, NPB], f32, tag="ps1",
                           name=f"ps1_{t}_{gi}")
            if t == 0:
                for s in range(NSLAB):
                    nc.tensor.matmul(ps1[:, s, 0:n], w1[0:XR, :],
                                     xt[0:XR, s, cs], start=True, stop=True)
                nc.vector.tensor_copy(m1t[:, :, cs], ps1[0:M1, :, 0:n])
            else:
                for s in range(NSLAB):
                    nc.tensor.matmul(ps1[:, s, 0:n], w1[:], xt[:, s, cs],
                                     start=True, stop=False)
                    nc.tensor.matmul(ps1[:, s, 0:n], r1[:],
                                     s1t[0:M1, s, cs],
                                     start=False, stop=True)
                nc.vector.scalar_tensor_tensor(
                    m1t[:, :, cs], m1t[:, :, cs], BETA,
                    ps1[0:M1, :, 0:n], AOp.mult, AOp.add)
            nc.scalar.activation(
                s1t[0:M1, :, cs], m1t[:, :, cs], Act.Sign,
                bias=negone[0:M1, :])

        def l2(t, gi, c0, n, vop):
            cs = slice(c0, c0 + n)
            nc.tensor.matmul(v[:, cs], w2a[:, t, :], s1t[:, 0, cs],
                             start=(t == 0), stop=False)
            nc.tensor.matmul(v[:, cs], w2b[:, t, :], s1t[:, 1, cs],
                             start=False, stop=(t == 0))
            if t > 0:
                nc.tensor.matmul(v[:, cs], r2[:, t, :], s2t[:, cs],
                                 start=False, stop=True)
            # staging = bf16(v - thr_t); spike sign preserved exactly
            if gi % 2 == 0:
                nc.scalar.activation(vop[:, cs], v[:, cs], Act.Copy,
                                     bias=-float(thr_t[t]))
            else:
                nc.vector.tensor_scalar(vop[:, cs], v[:, cs],
                                        -float(thr_t[t]), None, AOp.add)
            if t < T - 1:
                # s2 = (v > thr_t) == (staging > 0), from bf16 staging
                nc.gpsimd.tensor_scalar(s2t[:, cs], vop[:, cs], 0.0, None,
                                        AOp.is_gt)

        for t in range(T):
            if t + 2 < T:
                xdma(t + 2)
            vop = vo[t % 2]
            sched = [("l1", 0), ("l1", 1), ("l2", 0), ("l1", 2), ("l2", 1),
                     ("l1", 3), ("l2", 2), ("l1", 4), ("l2", 3), ("l1", 5),
                     ("l2", 4), ("l2", 5)]
            for kind, gi in sched:
                c0, n = groups[gi]
                if kind == "l1":
                    l1(t, gi, c0, n)
                else:
                    l2(t, gi, c0, n, vop)
            nc.sync.dma_start(out_d[t], vop[:])

    if split_waits:
        _split_multi_waits(nc)
    return nc


def prep_core_x(xpad, c):
    xc = xpad[:, c * BC : (c + 1) * BC, :]
    xc = xc.reshape(T, NSLAB, NBL, NCOLS, NI)
    # -> [T, (bl,i)=84, slab, NCOLS]
    xc = np.ascontiguousarray(xc.transpose(0, 2, 4, 1, 3)).reshape(
        T, XR, NSLAB, NCOLS)
    return xc


def unpack_mem(res_c):
    m2 = res_c["mem2"]  # [T, 126, NCOLS]
    v = m2.reshape(T, NSLAB, NBL, NO, NCOLS).transpose(0, 1, 2, 4, 3)
    return np.ascontiguousarray(v).reshape(T, BC, NO)


VERSION = 5


def build_for_bench(x, w1, w2):
    """Build (nc, in_maps) for external timing harnesses."""
    xpad = np.zeros((T, BPAD, NI), dtype=np.float32)
    xpad[:, :B_FULL] = np.asarray(x, np.float32)
    with ThreadPoolExecutor(8) as ex:
        xs = list(ex.map(lambda c: prep_core_x(xpad, c), range(NCORES)))
    onesv = np.ones((1, NSLAB * NCOLS), np.float32)
    if VERSION == 6:
        nc = build_nc_v6()
        w1f, r1, w2t, r2t, thr = make_weights_v6(w1, w2)
        w2a = np.ascontiguousarray(w2t[0].transpose(1, 0, 2))
        w2b = np.ascontiguousarray(w2t[1].transpose(1, 0, 2))
        r2s = np.ascontiguousarray(r2t.transpose(1, 0, 2))
        in_maps = [
            {"x": xs[c], "w1": w1f, "r1": r1, "w2a": w2a, "w2b": w2b,
             "r2": r2s, "ones": onesv}
            for c in range(NCORES)
        ]
    else:
        nc = build_nc_v5(**V5_OPTS)
        w1f, r1, w2f, r2, bI = make_weights(w1, w2)
        in_maps = [
            {"x": xs[c], "w1": w1f, "r1": r1, "w2a": w2f[0], "w2b": w2f[1],
             "r2": r2, "ones": onesv}
            for c in range(NCORES)
        ]
        if V5_OPTS.get("m2_path", "dve") != "dve":
            for m in in_maps:
                m["bI"] = bI
    return nc, in_maps


def kernel(**inputs):
    x = np.asarray(inputs["x"], dtype=np.float32)
    w1 = np.asarray(inputs["w1"], dtype=np.float32)
    w2 = np.asarray(inputs["w2"], dtype=np.float32)

    from concourse.bass_utils import run_bass_kernel_spmd

    xpad = np.zeros((T, BPAD, NI), dtype=np.float32)
    xpad[:, :B_FULL] = x
    with ThreadPoolExecutor(8) as ex:
        xs = list(ex.map(lambda c: prep_core_x(xpad, c), range(NCORES)))
    onesv = np.ones((1, NSLAB * NCOLS), np.float32)

    if VERSION == 6:
        nc = build_nc_v6()
        w1f, r1, w2t, r2t, thr = make_weights_v6(w1, w2)
        w2a = np.ascontiguousarray(w2t[0].transpose(1, 0, 2))
        w2b = np.ascontiguousarray(w2t[1].transpose(1, 0, 2))
        r2s = np.ascontiguousarray(r2t.transpose(1, 0, 2))
        in_maps = [
            {"x": xs[c], "w1": w1f, "r1": r1, "w2a": w2a, "w2b": w2b,
             "r2": r2s, "ones": onesv}
            for c in range(NCORES)
        ]
    else:
        nc = build_nc_v5(**V5_OPTS)
        w1f, r1, w2f, r2, bI = make_weights(w1, w2)
        in_maps = [
            {"x": xs[c], "w1": w1f, "r1": r1, "w2a": w2f[0], "w2b": w2f[1],
             "r2": r2, "ones": onesv}
            for c in range(NCORES)
        ]
        if V5_OPTS.get("m2_path", "dve") != "dve":
            for m in in_maps:
                m["bI"] = bI

    import time as _time
    _t0 = _time.time()
    res = run_bass_kernel_spmd(nc, in_maps, list(range(NCORES))).results
    print(f"[kernel] device compile+run {_time.time()-_t0:.1f}s", flush=True)

    if VERSION == 6:
        # vout = bf16(v - beta^-t); mem2 = beta^t*v = beta^t*vout + 1;
        # spk2 = (vout > 0) reproduces the device threshold bit-exactly.
        bt = (np.float64(BETA) ** np.arange(T)).astype(np.float32)
        spk2 = np.empty((T, BPAD, NO), dtype=np.float32)
        mem2 = np.empty((T, BPAD, NO), dtype=np.float32)

        def fill(c):
            vout = res[c]["vout"]  # [T, 126, NCOLS] bf16
            vo = np.asarray(vout, np.float32).reshape(
                T, NSLAB, NBL, NO, NCOLS).transpose(0, 1, 2, 4, 3)
            vo = np.ascontiguousarray(vo).reshape(T, BC, NO)
            sl = slice(c * BC, (c + 1) * BC)
            spk2[:, sl] = (vo > 0.0)
            mem2[:, sl] = vo * bt[:, None, None] + np.float32(1.0)

        with ThreadPoolExecutor(8) as ex:
            list(ex.map(fill, range(NCORES)))
        return spk2[:, :B_FULL], mem2[:, :B_FULL]

    mem2 = np.empty((T, BPAD, NO), dtype=np.float32)

    def fill(c):
        mem2[:, c * BC : (c + 1) * BC] = unpack_mem(res[c])

    with ThreadPoolExecutor(8) as ex:
        list(ex.map(fill, range(NCORES)))
    mem2 = mem2[:, :B_FULL]
    spk2 = (mem2 > THR).astype(np.float32)
    return spk2, mem2


# revision 55
# speedup vs baseline: 1.4506x; 1.4506x over previous
"""Spiking-net forward (2-layer LIF, subtract reset) on 8 TRN2 cores,
pure data-parallel over the batch.

v8 (final): all-fp16 matmul path, slab-major 2D tiles, engines balanced
at ~95-112us each per core (NTFF-measured ~160us total, vs ~505us for
the session-start baseline):

  per step t (threshold 1, decay beta; spikes coded +-1):
    ps1 = W1@[x;1] + R1(-I/2)@s1_prev        [PE fp16, psum fp32]
    m1  = beta*m1 + ps1                      [DVE, fp32 exact]
    s1  = Sign(m1 - 1)                       [ACT -> fp16 +-1]
    ps2 = W2a@[s1;1] + W2b@[s1;1] + R2@[s2;1]  [PE fp16]
    m2  = beta*m2 + ps2  -> DRAM fp32        [DVE; ping-pong + lazy DMA]
    s2  = Sign(m2 - 1)                       [ACT]

  - x and W1/W2' in fp16 (q10/q11): emulated end-to-end error 1.22e-2,
    measured 1.31e-2 (gate 2e-2). Reset matrices/spikes are exact fp16.
  - spk2 is not computed on device: host thresholds the exact fp32 mem2,
    bit-identical to the device's own Sign decisions.
  - gpsimd (Pool) carries no tensor ops (measured ~15us per op, 10x the
    cost model) - only small init DMAs.
  - 20 dummy matmuls at start + fillers in steps 0-3 hold the PE HAM
    clock-gate at K=8/8 (2.4 GHz); without them the PE runs its first
    ~60us at 1.2 GHz.
  - layout per core: 2 slabs x 21 lanes x 2978 cols; x rows 4*bl+i,
    l1 rows 5*bl+h (105), l2 pair-packs slabs into 126 = 2*63 rows.
"""

import numpy as np
from contextlib import ExitStack
from concurrent.futures import ThreadPoolExecutor

T = 10
NI, NH, NO = 4, 5, 3
BETA = 0.95
THR = 1.0
B_FULL = 1_000_000
NCORES = 8

NBL = 21
NSLAB = 2
NCOLS = 2978
NPB = 512
BC = NSLAB * NBL * NCOLS  # 125,076
BPAD = BC * NCORES        # 1,000,608

XR = NBL * NI             # 84
M1 = NBL * NH             # 105
M2 = NBL * NO             # 63
M2P = 2 * M2              # 126

V5_OPTS = dict(loop_split=True, l2w=1024, m2_path="dve", x16=True)


def make_weights16(w1, w2):
    """All-fp16 weights: hi/lo pairs for W1 and W2' (combined ~2^-21, so
    the only end-to-end error source is fp16(x) quantization ~q10)."""
    w1f, r1, w2f, r2, _ = make_weights(w1, w2)

    def hl(a):
        h = a.astype(np.float16)
        l = (a - h.astype(np.float32)).astype(np.float16)
        return h, l

    w1h, w1l = hl(w1f)
    w2ah, w2al = hl(w2f[0])
    w2bh, w2bl = hl(w2f[1])
    return (w1h, w1l, r1.astype(np.float16), w2ah, w2al, w2bh, w2bl,
            r2.astype(np.float16))


def make_weights(w1, w2):
    w1 = np.asarray(w1, np.float32)
    w2 = np.asarray(w2, np.float32)
    # W1 [85, 128]: rows (bl,i) -> w1[h,i] at col 5bl+h; ones row -> -1/2
    w1f = np.zeros((XR + 1, 128), np.float32)
    for bl in range(NBL):
        for i in range(NI):
            for h in range(NH):
                w1f[4 * bl + i, 5 * bl + h] = w1[h, i]
    w1f[XR, 0:M1] = -0.5
    # R1 [105, 128]: -I/2 on +-1 spikes
    r1 = np.zeros((M1, 128), np.float32)
    r1[:, 0:M1] = -0.5 * np.eye(M1, dtype=np.float32)
    # W2' per slab [106, 126]: w2/2 at [5bl+h, 63s+3bl+o]; bias row sum(w2)/2
    w2f = np.zeros((2, M1 + 1, M2P), np.float32)
    for s in range(2):
        for bl in range(NBL):
            for h in range(NH):
                for o in range(NO):
                    w2f[s, 5 * bl + h, 63 * s + 3 * bl + o] = w2[o, h] / 2.0
        for bl in range(NBL):
            for o in range(NO):
                w2f[s, M1, 63 * s + 3 * bl + o] = float(
                    w2[o].astype(np.float64).sum()) / 2.0
    # R2 [127, 126]: -(s+1)/2 reset on +-1 spikes: -I/2 plus ones-row -1/2
    r2 = np.zeros((M2P + 1, M2P), np.float32)
    r2[0:M2P] = -0.5 * np.eye(M2P, dtype=np.float32)
    r2[M2P, :] = -0.5
    # beta*I for the PE-side m2 decay (m2_path != 'dve')
    bI = (BETA * np.eye(M2P)).astype(np.float32)
    return w1f, r1, w2f, r2, bI


def _split_multi_waits(nc):
    """Walrus codegen for compute-engine ISA slots accepts only ONE sync-wait
    command per instruction. Tile sometimes attaches 2+ (e.g. own-engine sem +
    a DMA-completion lane). Hoist the extras onto pure-sync EventSemaphore
    instructions inserted just before, on the same engine queue."""
    import concourse.mybir as mybir

    for f in nc.m.functions:
        for blk in f.blocks:
            out = []
            for ins in blk.instructions:
                si = ins.sync_info
                if (
                    si is not None
                    and len(si.on_wait) > 1
                    and not isinstance(ins, mybir.InstEventSemaphore)
                ):
                    waits = list(si.on_wait)
                    for j, w in enumerate(waits[:-1]):
                        out.append(
                            mybir.InstEventSemaphore(
                                name=f"{ins.name}-ws{j}",
                                engine=ins.engine,
                                ins=[],
                                outs=[],
                                sync_info=mybir.SyncInfo(
                                    on_wait=[w], on_update=[]
                                ),
                            )
                        )
                    ins.sync_info = mybir.SyncInfo(
                        on_wait=[waits[-1]], on_update=list(si.on_update)
                    )
                out.append(ins)
            blk.instructions = out


def build_nc_v5(split_waits=True, loop_split=True, l2w=1024, m2_path="dve",
                x16=False, reps=1):
    """m2_path: 'dve' = exact fp32 stt on DVE; 'act'/'pool' = beta*I matmul
    into psum + Copy on ACT/Pool (m2 state then lives in f32r)."""
    import concourse.bass as bass
    import concourse.mybir as mybir
    from concourse.tile import TileContext

    f32 = mybir.dt.float32
    f32r = mybir.dt.float32r
    Act = mybir.ActivationFunctionType
    AOp = mybir.AluOpType

    def mkgroups(w):
        out, c0 = [], 0
        while c0 < NCOLS:
            n = min(w, NCOLS - c0)
            out.append((c0, n))
            c0 += n
        return out

    groups = mkgroups(NPB)
    groups2 = mkgroups(l2w) if loop_split else groups
    m2_f32r = m2_path != "dve"
    m2_dt_np = np.float32

    fp16 = mybir.dt.float16
    x_dt = fp16 if x16 else f32r

    nc = bass.Bass()
    x_d = nc.declare_dram_parameter("x", [T, XR, NSLAB, NCOLS], x_dt,
                                    isOutput=False)
    w1_d = nc.declare_dram_parameter("w1", [XR + 1, 128], x_dt, isOutput=False)
    r1_d = nc.declare_dram_parameter("r1", [M1, 128], f32r, isOutput=False)
    w2a_d = nc.declare_dram_parameter("w2a", [M1 + 1, M2P], f32r, isOutput=False)
    w2b_d = nc.declare_dram_parameter("w2b", [M1 + 1, M2P], f32r, isOutput=False)
    r2_d = nc.declare_dram_parameter("r2", [M2P + 1, M2P], f32r,
                                     isOutput=False)
    if m2_f32r:
        bI_d = nc.declare_dram_parameter("bI", [M2P, M2P], f32r,
                                         isOutput=False)
    ones_d = nc.declare_dram_parameter("ones", [1, NSLAB * NCOLS], f32r,
                                       isOutput=False)
    if x16:
        ones16_d = nc.declare_dram_parameter("ones16", [1, NSLAB * NCOLS],
                                             fp16, isOutput=False)
    m2_dt = f32r if m2_f32r else f32
    mem_d = nc.declare_dram_parameter("mem2", [T, M2P, NCOLS], m2_dt,
                                      isOutput=True)

    with ExitStack() as ctx:
        tc = ctx.enter_context(TileContext(nc))
        wp = ctx.enter_context(tc.tile_pool(name="wp", bufs=1))
        st = ctx.enter_context(tc.tile_pool(name="st", bufs=1))
        xp = ctx.enter_context(tc.tile_pool(name="xp", bufs=1))
        psa = ctx.enter_context(tc.tile_pool(
            name="psa", bufs=2 if loop_split else 3, space="PSUM"))
        psb = ctx.enter_context(tc.tile_pool(name="psb", bufs=2, space="PSUM"))

        negone = wp.tile([128, 1], f32, tag="negone")
        nc.vector.memset(negone[:], -1.0)
        w1 = wp.tile([XR + 1, 128], x_dt, tag="w1")
        r1 = wp.tile([M1, 128], f32r, tag="r1")
        w2a = wp.tile([M1 + 1, M2P], f32r, tag="w2a")
        w2b = wp.tile([M1 + 1, M2P], f32r, tag="w2b")
        r2 = wp.tile([M2P + 1, M2P], f32r, tag="r2")
        wdmas = [(w1, w1_d), (r1, r1_d), (w2a, w2a_d), (w2b, w2b_d),
                 (r2, r2_d)]
        if m2_f32r:
            bI = wp.tile([M2P, M2P], f32r, tag="bI")
            wdmas.append((bI, bI_d))
        for tl, dr in wdmas:
            nc.sync.dma_start(tl[:], dr[:])

        # persistent state; no zero-init needed: step 0 skips the reset
        # matmuls (resets are zero) and copies psum instead of stt.
        m1t = st.tile([M1, NSLAB, NCOLS], f32, tag="m1", name="m1")
        s1t = st.tile([M1 + 1, NSLAB, NCOLS], f32r, tag="s1", name="s1")
        s2t = st.tile([M2P + 1, NCOLS], f32r, tag="s2", name="s2")
        m2t = [st.tile([M2P, NCOLS], m2_dt, tag=f"m2_{k}", name=f"m2_{k}")
               for k in range(2)]
        # x ring: 3 step-tiles; row 84 = ones (set once via DMA)
        xs = [xp.tile([XR + 1, NSLAB, NCOLS], x_dt, tag=f"x_{r}",
                      name=f"x_{r}") for r in range(3)]

        nc.sync.dma_start(s1t[M1:M1 + 1, :, :], ones_d[:])
        nc.sync.dma_start(s2t[M2P:M2P + 1, :], ones_d[:, 0:NCOLS])
        xones_d = ones16_d if x16 else ones_d
        for r in range(3):
            nc.sync.dma_start(xs[r][XR:XR + 1, :, :], xones_d[:])

        def xdma(t):
            nc.sync.dma_start(xs[t % 3][0:XR, :, :], x_d[t])

        uid = [0]

        def l1(t, gi, c0, n):
            xt = xs[t % 3]
            cs = slice(c0, c0 + n)
            uid[0] += 1
            ps1 = psa.tile([128, NSLAB, NPB], f32, tag="ps1",
                           name=f"ps1_{uid[0]}")
            if t == 0:
                # reset1 = 0: no ones row (-1/2) and no r1 matmul
                for s in range(NSLAB):
                    nc.tensor.matmul(ps1[:, s, 0:n], w1[0:XR, :],
                                     xt[0:XR, s, cs], start=True, stop=True)
                nc.vector.tensor_copy(m1t[:, :, cs], ps1[0:M1, :, 0:n])
            else:
                for s in range(NSLAB):
                    nc.tensor.matmul(ps1[:, s, 0:n], w1[:], xt[:, s, cs],
                                     start=True, stop=False)
                    nc.tensor.matmul(ps1[:, s, 0:n], r1[:],
                                     s1t[0:M1, s, cs],
                                     start=False, stop=True)
                # m1 = beta*m1 + ps1  (both slabs in one op)
                nc.vector.scalar_tensor_tensor(
                    m1t[:, :, cs], m1t[:, :, cs], BETA,
                    ps1[0:M1, :, 0:n], AOp.mult, AOp.add)
            # s1 = Sign(m1 - 1) in f32r
            nc.scalar.activation(
                s1t[0:M1, :, cs], m1t[:, :, cs], Act.Sign,
                bias=negone[0:M1, :])

        def l2(t, gi, c0, n, mo, mp):
            cs = slice(c0, c0 + n)
            uid[0] += 1
            ps2 = psb.tile([M2P, l2w], f32, tag="ps2", name=f"ps2_{uid[0]}")

            def mm(w, rhs_fn, start, stop):
                o = 0
                while o < n:
                    k = min(512, n - o)
                    nc.tensor.matmul(ps2[:, o:o + k], w,
                                     rhs_fn(c0 + o, c0 + o + k),
                                     start=start, stop=stop)
                    o += k

            mm(w2a[:], lambda a, b: s1t[:, 0, a:b], True, False)
            mm(w2b[:], lambda a, b: s1t[:, 1, a:b], False, t == 0)
            if t > 0:
                mm(r2[:], lambda a, b: s2t[:, a:b], False, not m2_f32r)
                if m2_f32r:
                    mm(bI[:], lambda a, b: mo[:, a:b], False, True)
            # m2 = beta*m2_prev + ps2  (ping-pong state for lazy DMA)
            if m2_path == "dve":
                if t == 0:
                    nc.vector.tensor_copy(mp[:, cs], ps2[:, 0:n])
                else:
                    nc.vector.scalar_tensor_tensor(
                        mp[:, cs], mo[:, cs], BETA, ps2[:, 0:n],
                        AOp.mult, AOp.add)
            elif m2_path == "act":
                nc.scalar.activation(mp[:, cs], ps2[:, 0:n], Act.Copy)
            else:
                nc.gpsimd.tensor_copy(mp[:, cs], ps2[:, 0:n])
            # s2 = Sign(m2 - 1) in {-1,+1} f32r (gpsimd is ~10x too slow
            # for tensor ops on real HW; keep everything off it)
            if t < T - 1:
                nc.scalar.activation(s2t[0:M2P, cs], mp[:, cs], Act.Sign,
                                     bias=negone[0:M2P, :])

        for rep in range(reps):
            xdma(0)
            xdma(1)
            for t in range(T):
                if t + 2 < T:
                    xdma(t + 2)
                mo, mp = m2t[t % 2], m2t[(t + 1) % 2]
                if loop_split:
                    for gi, (c0, n) in enumerate(groups):
                        l1(t, gi, c0, n)
                    for gi, (c0, n) in enumerate(groups2):
                        l2(t, gi, c0, n, mo, mp)
                else:
                    for gi, (c0, n) in enumerate(groups):
                        l1(t, gi, c0, n)
                        l2(t, gi, c0, n, mo, mp)
                nc.sync.dma_start(mem_d[t], mp[:])

    if split_waits:
        _split_multi_waits(nc)
    return nc


def build_nc_v7(split_waits=True, l2w=1024, reps=1):
    """All-fp16 matmul path: x fp16 (q10, the only error source), W1/W2'
    as fp16 hi/lo pairs (~2^-21), r1/r2/spikes exact in fp16. fp16 runs
    1 cyc/col on the PE (f32r needs 2) and allows 1024-col moving APs, so
    one l1 matmul covers both slabs via a 3D access pattern."""
    import concourse.bass as bass
    import concourse.mybir as mybir
    from concourse.tile import TileContext

    f32 = mybir.dt.float32
    fp16 = mybir.dt.float16
    Act = mybir.ActivationFunctionType
    AOp = mybir.AluOpType

    def mkgroups(w):
        out, c0 = [], 0
        while c0 < NCOLS:
            n = min(w, NCOLS - c0)
            out.append((c0, n))
            c0 += n
        return out

    groups = mkgroups(NPB)
    groups2 = mkgroups(l2w)

    nc = bass.Bass()
    x_d = nc.declare_dram_parameter("x", [T, XR, NSLAB, NCOLS], fp16,
                                    isOutput=False)
    w1h_d = nc.declare_dram_parameter("w1h", [XR + 1, 128], fp16,
                                      isOutput=False)
    w1l_d = nc.declare_dram_parameter("w1l", [XR, 128], fp16, isOutput=False)
    r1_d = nc.declare_dram_parameter("r1", [M1, 128], fp16, isOutput=False)
    w2ah_d = nc.declare_dram_parameter("w2ah", [M1 + 1, M2P], fp16,
                                       isOutput=False)
    w2al_d = nc.declare_dram_parameter("w2al", [M1 + 1, M2P], fp16,
                                       isOutput=False)
    w2bh_d = nc.declare_dram_parameter("w2bh", [M1 + 1, M2P], fp16,
                                       isOutput=False)
    w2bl_d = nc.declare_dram_parameter("w2bl", [M1 + 1, M2P], fp16,
                                       isOutput=False)
    r2_d = nc.declare_dram_parameter("r2", [M2P + 1, M2P], fp16,
                                     isOutput=False)
    ones_d = nc.declare_dram_parameter("ones16", [1, NSLAB * NCOLS], fp16,
                                       isOutput=False)
    mem_d = nc.declare_dram_parameter("mem2", [T, M2P, NCOLS], f32,
                                      isOutput=True)

    with ExitStack() as ctx:
        tc = ctx.enter_context(TileContext(nc))
        wp = ctx.enter_context(tc.tile_pool(name="wp", bufs=1))
        st = ctx.enter_context(tc.tile_pool(name="st", bufs=1))
        xp = ctx.enter_context(tc.tile_pool(name="xp", bufs=1))
        psa = ctx.enter_context(tc.tile_pool(name="psa", bufs=2, space="PSUM"))
        psb = ctx.enter_context(tc.tile_pool(name="psb", bufs=2, space="PSUM"))

        negone = wp.tile([128, 1], f32, tag="negone")
        nc.vector.memset(negone[:], -1.0)
        w1h = wp.tile([XR + 1, 128], fp16, tag="w1h")
        w1l = wp.tile([XR, 128], fp16, tag="w1l")
        r1 = wp.tile([M1, 128], fp16, tag="r1")
        w2ah = wp.tile([M1 + 1, M2P], fp16, tag="w2ah")
        w2al = wp.tile([M1 + 1, M2P], fp16, tag="w2al")
        w2bh = wp.tile([M1 + 1, M2P], fp16, tag="w2bh")
        w2bl = wp.tile([M1 + 1, M2P], fp16, tag="w2bl")
        r2 = wp.tile([M2P + 1, M2P], fp16, tag="r2")
        for tl, dr in ((w1h, w1h_d), (w1l, w1l_d), (r1, r1_d),
                       (w2ah, w2ah_d), (w2al, w2al_d), (w2bh, w2bh_d),
                       (w2bl, w2bl_d), (r2, r2_d)):
            nc.sync.dma_start(tl[:], dr[:])

        m1t = st.tile([M1, NSLAB, NCOLS], f32, tag="m1", name="m1")
        s1t = st.tile([M1 + 1, NSLAB, NCOLS], fp16, tag="s1", name="s1")
        s2t = st.tile([M2P + 1, NCOLS], fp16, tag="s2", name="s2")
        m2t = [st.tile([M2P, NCOLS], f32, tag=f"m2_{k}", name=f"m2_{k}")
               for k in range(2)]
        xs = [xp.tile([XR + 1, NSLAB, NCOLS], fp16, tag=f"x_{r}",
                      name=f"x_{r}") for r in range(3)]

        nc.sync.dma_start(s1t[M1:M1 + 1, :, :], ones_d[:])
        nc.sync.dma_start(s2t[M2P:M2P + 1, :], ones_d[:, 0:NCOLS])
        for r in range(3):
            nc.sync.dma_start(xs[r][XR:XR + 1, :, :], ones_d[:])

        def xdma(t):
            nc.sync.dma_start(xs[t % 3][0:XR, :, :], x_d[t])

        uid = [0]

        def l1(t, gi, c0, n):
            xt = xs[t % 3]
            cs = slice(c0, c0 + n)
            uid[0] += 1
            ps1 = psa.tile([128, NSLAB, NPB], f32, tag="ps1",
                           name=f"ps1_{uid[0]}")
            if t == 0:
                for s in range(NSLAB):
                    nc.tensor.matmul(ps1[:, s, 0:n], w1h[0:XR, :],
                                     xt[0:XR, s, cs], start=True, stop=False)
                    nc.tensor.matmul(ps1[:, s, 0:n], w1l[:],
                                     xt[0:XR, s, cs], start=False, stop=True)
                nc.vector.tensor_copy(m1t[:, :, cs], ps1[0:M1, :, 0:n])
            else:
                for s in range(NSLAB):
                    nc.tensor.matmul(ps1[:, s, 0:n], w1h[:], xt[:, s, cs],
                                     start=True, stop=False)
                    nc.tensor.matmul(ps1[:, s, 0:n], w1l[:],
                                     xt[0:XR, s, cs], start=False, stop=False)
                    nc.tensor.matmul(ps1[:, s, 0:n], r1[:],
                                     s1t[0:M1, s, cs],
                                     start=False, stop=True)
                nc.vector.scalar_tensor_tensor(
                    m1t[:, :, cs], m1t[:, :, cs], BETA,
                    ps1[0:M1, :, 0:n], AOp.mult, AOp.add)
            nc.scalar.activation(
                s1t[0:M1, :, cs], m1t[:, :, cs], Act.Sign,
                bias=negone[0:M1, :])

        def l2(t, gi, c0, n, mo, mp):
            cs = slice(c0, c0 + n)
            uid[0] += 1
            ps2 = psb.tile([M2P, l2w], f32, tag="ps2", name=f"ps2_{uid[0]}")

            def mm(w, rhs_fn, start, stop):
                o = 0
                while o < n:
                    k = min(512, n - o)
                    nc.tensor.matmul(ps2[:, o:o + k], w,
                                     rhs_fn(c0 + o, c0 + o + k),
                                     start=start, stop=stop)
                    o += k

            mm(w2ah[:], lambda a, b: s1t[:, 0, a:b], True, False)
            mm(w2al[:], lambda a, b: s1t[:, 0, a:b], False, False)
            mm(w2bh[:], lambda a, b: s1t[:, 1, a:b], False, False)
            mm(w2bl[:], lambda a, b: s1t[:, 1, a:b], False, t == 0)
            if t > 0:
                mm(r2[:], lambda a, b: s2t[:, a:b], False, True)
            if t == 0:
                nc.vector.tensor_copy(mp[:, cs], ps2[:, 0:n])
            else:
                nc.vector.scalar_tensor_tensor(
                    mp[:, cs], mo[:, cs], BETA, ps2[:, 0:n],
                    AOp.mult, AOp.add)
            if t < T - 1:
                nc.scalar.activation(s2t[0:M2P, cs], mp[:, cs], Act.Sign,
                                     bias=negone[0:M2P, :])

        for rep in range(reps):
            xdma(0)
            xdma(1)
            for t in range(T):
                if t + 2 < T:
                    xdma(t + 2)
                mo, mp = m2t[t % 2], m2t[(t + 1) % 2]
                for gi, (c0, n) in enumerate(groups):
                    l1(t, gi, c0, n)
                for gi, (c0, n) in enumerate(groups2):
                    l2(t, gi, c0, n, mo, mp)
                nc.sync.dma_start(mem_d[t], mp[:])

    if split_waits:
        _split_multi_waits(nc)
    return nc


def make_weights_v6(w1, w2):
    """Layer-2 state is tracked as v(t) = mem2(t)/beta^t, accumulated in
    PSUM across all T steps (PE start=False). The decay folds into
    t-scaled stationaries: W2'_t = (w2/2)*beta^-t (+bias row), r2_t =
    -beta^-t * I. Spike threshold for v is thr_t = beta^-t."""
    w1f, r1, w2f, r2_, bI_ = make_weights(w1, w2)
    sc = (1.0 / np.float64(BETA) ** np.arange(T)).astype(np.float64)
    w2t = np.zeros((2, T, M1 + 1, M2P), np.float32)
    r2t = np.zeros((T, M2P, M2P), np.float32)
    for t in range(T):
        w2t[0, t] = (w2f[0].astype(np.float64) * sc[t]).astype(np.float32)
        w2t[1, t] = (w2f[1].astype(np.float64) * sc[t]).astype(np.float32)
        r2t[t] = (-sc[t] * np.eye(M2P)).astype(np.float32)
    thr = sc.astype(np.float32)  # thr_t = beta^-t * THR (THR=1)
    return w1f, r1, w2t, r2t, thr


def build_nc_v6(split_waits=True):
    import concourse.bass as bass
    import concourse.mybir as mybir
    from concourse.tile import TileContext

    f32 = mybir.dt.float32
    f32r = mybir.dt.float32r
    bf16 = mybir.dt.bfloat16
    Act = mybir.ActivationFunctionType
    AOp = mybir.AluOpType

    groups = []
    c0 = 0
    while c0 < NCOLS:
        n = min(NPB, NCOLS - c0)
        groups.append((c0, n))
        c0 += n
    NG = len(groups)

    # host-exact beta^-t thresholds (float32)
    thr_t = [np.float32(1.0 / np.float64(BETA) ** t) for t in range(T)]

    nc = bass.Bass()
    x_d = nc.declare_dram_parameter("x", [T, XR, NSLAB, NCOLS], f32r,
                                    isOutput=False)
    w1_d = nc.declare_dram_parameter("w1", [XR + 1, 128], f32r, isOutput=False)
    r1_d = nc.declare_dram_parameter("r1", [M1, 128], f32r, isOutput=False)
    w2a_d = nc.declare_dram_parameter("w2a", [M1 + 1, T, M2P], f32r,
                                      isOutput=False)
    w2b_d = nc.declare_dram_parameter("w2b", [M1 + 1, T, M2P], f32r,
                                      isOutput=False)
    r2_d = nc.declare_dram_parameter("r2", [M2P, T, M2P], f32r,
                                     isOutput=False)
    ones_d = nc.declare_dram_parameter("ones", [1, NSLAB * NCOLS], f32r,
                                       isOutput=False)
    out_d = nc.declare_dram_parameter("vout", [T, M2P, NCOLS], bf16,
                                      isOutput=True)

    with ExitStack() as ctx:
        tc = ctx.enter_context(TileContext(nc))
        wp = ctx.enter_context(tc.tile_pool(name="wp", bufs=1))
        st = ctx.enter_context(tc.tile_pool(name="st", bufs=1))
        xp = ctx.enter_context(tc.tile_pool(name="xp", bufs=1))
        psa = ctx.enter_context(tc.tile_pool(name="psa", bufs=1, space="PSUM"))
        psv = ctx.enter_context(tc.tile_pool(name="psv", bufs=1, space="PSUM"))

        negone = wp.tile([128, 1], f32, tag="negone")
        nc.vector.memset(negone[:], -1.0)
        w1 = wp.tile([XR + 1, 128], f32r, tag="w1")
        r1 = wp.tile([M1, 128], f32r, tag="r1")
        w2a = wp.tile([M1 + 1, T, M2P], f32r, tag="w2a")
        w2b = wp.tile([M1 + 1, T, M2P], f32r, tag="w2b")
        r2 = wp.tile([M2P, T, M2P], f32r, tag="r2")
        for tl, dr in ((w1, w1_d), (r1, r1_d), (w2a, w2a_d), (w2b, w2b_d),
                       (r2, r2_d)):
            nc.sync.dma_start(tl[:], dr[:])

        m1t = st.tile([M1, NSLAB, NCOLS], f32, tag="m1", name="m1")
        s1t = st.tile([M1 + 1, NSLAB, NCOLS], f32r, tag="s1", name="s1")
        s2t = st.tile([M2P, NCOLS], f32r, tag="s2", name="s2")
        # bf16 staging of (v - thr_t): spike sign exact under rounding
        vo = [st.tile([M2P, NCOLS], bf16, tag=f"vo_{k}", name=f"vo_{k}")
              for k in range(2)]
        # v = mem2/beta^t accumulated in PSUM across all steps (6 banks)
        v = psv.tile([M2P, NCOLS], f32, tag="v", name="v")
        xs = [xp.tile([XR + 1, NSLAB, NCOLS], f32r, tag=f"x_{r}",
                      name=f"x_{r}") for r in range(3)]

        nc.sync.dma_start(s1t[M1:M1 + 1, :, :], ones_d[:])
        for r in range(3):
            nc.sync.dma_start(xs[r][XR:XR + 1, :, :], ones_d[:])

        def xdma(t):
            nc.sync.dma_start(xs[t % 3][0:XR, :, :], x_d[t])

        xdma(0)
        xdma(1)

        def l1(t, gi, c0, n):
            xt = xs[t % 3]
            cs = slice(c0, c0 + n)
            ps1 = psa.tile([128, NSLAB, NPB], f32, tag="ps1",
                           name=f"ps1_{t}_{gi}")
            if t == 0:
                for s in range(NSLAB):
                    nc.tensor.matmul(ps1[:, s, 0:n], w1[0:XR, :],
                                     xt[0:XR, s, cs], start=True, stop=True)
                nc.vector.tensor_copy(m1t[:, :, cs], ps1[0:M1, :, 0:n])
            else:
                for s in range(NSLAB):
                    nc.tensor.matmul(ps1[:, s, 0:n], w1[:], xt[:, s, cs],
                                     start=True, stop=False)
                    nc.tensor.matmul(ps1[:, s, 0:n], r1[:],
                                     s1t[0:M1, s, cs],
                                     start=False, stop=True)
                nc.vector.scalar_tensor_tensor(
                    m1t[:, :, cs], m1t[:, :, cs], BETA,
                    ps1[0:M1, :, 0:n], AOp.mult, AOp.add)
            nc.scalar.activation(
                s1t[0:M1, :, cs], m1t[:, :, cs], Act.Sign,
                bias=negone[0:M1, :])

        def l2(t, gi, c0, n, vop):
            cs = slice(c0, c0 + n)
            nc.tensor.matmul(v[:, cs], w2a[:, t, :], s1t[:, 0, cs],
                             start=(t == 0), stop=False)
            nc.tensor.matmul(v[:, cs], w2b[:, t, :], s1t[:, 1, cs],
                             start=False, stop=(t == 0))
            if t > 0:
                nc.tensor.matmul(v[:, cs], r2[:, t, :], s2t[:, cs],
                                 start=False, stop=True)
            # staging = bf16(v - thr_t); spike sign preserved exactly
            if gi % 2 == 0:
                nc.scalar.activation(vop[:, cs], v[:, cs], Act.Copy,
                                     bias=-float(thr_t[t]))
            else:
                nc.vector.tensor_scalar(vop[:, cs], v[:, cs],
                                        -float(thr_t[t]), None, AOp.add)
            if t < T - 1:
                # s2 = (v > thr_t) == (staging > 0), from bf16 staging
                nc.gpsimd.tensor_scalar(s2t[:, cs], vop[:, cs], 0.0, None,
                                        AOp.is_gt)

        for t in range(T):
            if t + 2 < T:
                xdma(t + 2)
            vop = vo[t % 2]
            sched = [("l1", 0), ("l1", 1), ("l2", 0), ("l1", 2), ("l2", 1),
                     ("l1", 3), ("l2", 2), ("l1", 4), ("l2", 3), ("l1", 5),
                     ("l2", 4), ("l2", 5)]
            for kind, gi in sched:
                c0, n = groups[gi]
                if kind == "l1":
                    l1(t, gi, c0, n)
                else:
                    l2(t, gi, c0, n, vop)
            nc.sync.dma_start(out_d[t], vop[:])

    if split_waits:
        _split_multi_waits(nc)
    return nc


def prep_core_x(xpad, c):
    xc = xpad[:, c * BC : (c + 1) * BC, :]
    xc = xc.reshape(T, NSLAB, NBL, NCOLS, NI)
    # -> [T, (bl,i)=84, slab, NCOLS]
    xc = np.ascontiguousarray(xc.transpose(0, 2, 4, 1, 3)).reshape(
        T, XR, NSLAB, NCOLS)
    return xc


def unpack_mem(res_c):
    m2 = res_c["mem2"]  # [T, 126, NCOLS]
    v = m2.reshape(T, NSLAB, NBL, NO, NCOLS).transpose(0, 1, 2, 4, 3)
    return np.ascontiguousarray(v).reshape(T, BC, NO)


def make_weights8(w1, w2):
    """v8: W1 fp16 hi/lo; W2' single fp16; r1/r2 exact fp16."""
    w1f, r1, w2f, r2, _ = make_weights(w1, w2)
    w1h = w1f.astype(np.float16)
    w1l = (w1f - w1h.astype(np.float32)).astype(np.float16)
    return (w1h, w1l[0:XR], r1.astype(np.float16),
            w2f[0].astype(np.float16), w2f[1].astype(np.float16),
            r2.astype(np.float16))


def build_nc_v8(split_waits=True, reps=1):
    """All-fp16, slab-major 2D tiles, 1024-col matmuls (27 mm/step).
    x fp16 (q10) + W2' fp16 (q10) are the error sources (~1.03e-2 emulated);
    W1 is an fp16 hi/lo pair, spikes/resets exact. l1 units interleave
    slabs so each l2 group unblocks after two l1 units."""
    import concourse.bass as bass
    import concourse.mybir as mybir
    from concourse.tile import TileContext

    f32 = mybir.dt.float32
    fp16 = mybir.dt.float16
    Act = mybir.ActivationFunctionType
    AOp = mybir.AluOpType

    W = 1024
    NC2 = NSLAB * NCOLS

    def chunks(total):
        out, c0 = [], 0
        while c0 < total:
            n = min(W, total - c0)
            out.append((c0, n))
            c0 += n
        return out

    sl_chunks = chunks(NCOLS)           # per-slab l1 col chunks
    l2_chunks = chunks(NCOLS)           # l2 groups over NCOLS

    nc = bass.Bass()
    x_d = nc.declare_dram_parameter("x", [T, XR, NSLAB, NCOLS], fp16,
                                    isOutput=False)
    w1h_d = nc.declare_dram_parameter("w1h", [XR + 1, 128], fp16,
                                      isOutput=False)
    w1l_d = nc.declare_dram_parameter("w1l", [XR, 128], fp16, isOutput=False)
    r1_d = nc.declare_dram_parameter("r1", [M1, 128], fp16, isOutput=False)
    w2a_d = nc.declare_dram_parameter("w2a", [M1 + 1, M2P], fp16,
                                      isOutput=False)
    w2b_d = nc.declare_dram_parameter("w2b", [M1 + 1, M2P], fp16,
                                      isOutput=False)
    r2_d = nc.declare_dram_parameter("r2", [M2P + 1, M2P], fp16,
                                     isOutput=False)
    ones_d = nc.declare_dram_parameter("ones16", [1, NC2], fp16,
                                       isOutput=False)
    mem_d = nc.declare_dram_parameter("mem2", [T, M2P, NCOLS], f32,
                                      isOutput=True)

    with ExitStack() as ctx:
        tc = ctx.enter_context(TileContext(nc))
        wp = ctx.enter_context(tc.tile_pool(name="wp", bufs=1))
        st = ctx.enter_context(tc.tile_pool(name="st", bufs=1))
        xp = ctx.enter_context(tc.tile_pool(name="xp", bufs=1))
        psa = ctx.enter_context(tc.tile_pool(name="psa", bufs=2, space="PSUM"))
        psb = ctx.enter_context(tc.tile_pool(name="psb", bufs=2, space="PSUM"))

        negone = wp.tile([128, 1], f32, tag="negone")
        nc.vector.memset(negone[:], -1.0)
        w1h = wp.tile([XR + 1, 128], fp16, tag="w1h")
        w1l = wp.tile([XR, 128], fp16, tag="w1l")
        r1 = wp.tile([M1, 128], fp16, tag="r1")
        w2a = wp.tile([M1 + 1, M2P], fp16, tag="w2a")
        w2b = wp.tile([M1 + 1, M2P], fp16, tag="w2b")
        r2 = wp.tile([M2P + 1, M2P], fp16, tag="r2")
        weng = [nc.scalar, nc.gpsimd, nc.scalar, nc.gpsimd, nc.scalar,
                nc.gpsimd]
        for (tl, dr), eng in zip(((w1h, w1h_d), (w1l, w1l_d), (r1, r1_d),
                                  (w2a, w2a_d), (w2b, w2b_d), (r2, r2_d)),
                                 weng):
            eng.dma_start(tl[:], dr[:])

        # slab-major 2D state: col = slab*NCOLS + c
        m1t = st.tile([M1, NC2], f32, tag="m1", name="m1")
        s1t = st.tile([M1 + 1, NC2], fp16, tag="s1", name="s1")
        m1m = m1t[:].rearrange("p (s c) -> p s c", s=NSLAB)
        s1m = s1t[0:M1, :].rearrange("p (s c) -> p s c", s=NSLAB)
        s2t = st.tile([M2P + 1, NCOLS], fp16, tag="s2", name="s2")
        m2t = [st.tile([M2P, NCOLS], f32, tag=f"m2_{k}", name=f"m2_{k}")
               for k in range(2)]
        xs = [xp.tile([XR + 1, NC2], fp16, tag=f"x_{r}", name=f"x_{r}")
              for r in range(3)]

        nc.scalar.dma_start(s1t[M1:M1 + 1, :], ones_d[:])
        nc.gpsimd.dma_start(s2t[M2P:M2P + 1, :], ones_d[:, 0:NCOLS])
        for r in range(3):
            nc.gpsimd.dma_start(xs[r][XR:XR + 1, :], ones_d[:])

        # HAM warm-up: ~9us of dense dummy matmuls while the first x DMA
        # streams in. Without this the PE runs at K=4/8 (1.2 GHz) for the
        # first ~60us (measured via NTFF ham events).
        wsc = wp.tile([128, 512], fp16, tag="wsc")
        nc.vector.memset(wsc[:], 0.0)

        def xdma(t):
            # dest [84, slab-major 5956] <- src [84, 2, 2978]: same order
            nc.sync.dma_start(xs[t % 3][0:XR, :], x_d[t])

        uid = [0]

        def warm_fill(k):
            # dummy matmuls keep the PE HAM clock-gate at K=8/8; scratch
            # psum goes through the ring so buffer deps stay sound
            uid[0] += 1
            pw = psa.tile([128, W], f32, tag="ps1", name=f"pw_{uid[0]}")
            for _ in range(k):
                nc.tensor.matmul(pw[0:126, 0:512], wsc[0:128, 0:126],
                                 wsc[:, 0:512], start=True, stop=True)

        def l1(t, s, c0, n):
            xt = xs[t % 3]
            a, b = s * NCOLS + c0, s * NCOLS + c0 + n
            uid[0] += 1
            ps1 = psa.tile([128, W], f32, tag="ps1", name=f"ps1_{uid[0]}")

            def mm1(wt, rows, rhs_rows, start, stop):
                o = 0
                while o < n:
                    k = min(512, n - o)
                    nc.tensor.matmul(ps1[:, o:o + k], wt[0:rows, :],
                                     xt[0:rhs_rows, a + o:a + o + k],
                                     start=start, stop=stop)
                    o += k

            def mmr1(start, stop):
                o = 0
                while o < n:
                    k = min(512, n - o)
                    nc.tensor.matmul(ps1[:, o:o + k], r1[:],
                                     s1t[0:M1, a + o:a + o + k],
                                     start=start, stop=stop)
                    o += k

            if t == 0:
                mm1(w1h, XR, XR, True, True)
                nc.vector.tensor_copy(m1t[:, a:b], ps1[0:M1, 0:n])
            else:
                mm1(w1h, XR + 1, XR + 1, True, False)
                mmr1(False, True)
                nc.vector.scalar_tensor_tensor(
                    m1t[:, a:b], m1t[:, a:b], BETA,
                    ps1[0:M1, 0:n], AOp.mult, AOp.add)

        def l2(t, c0, n, mo, mp):
            cs = slice(c0, c0 + n)
            uid[0] += 1
            ps2 = psb.tile([M2P, W], f32, tag="ps2", name=f"ps2_{uid[0]}")

            def mm2(wt, rhs, base, start, stop):
                o = 0
                while o < n:
                    k = min(512, n - o)
                    nc.tensor.matmul(ps2[:, o:o + k], wt,
                                     rhs[:, base + o:base + o + k],
                                     start=start, stop=stop)
                    o += k

            mm2(w2a[:], s1t, c0, True, False)
            mm2(w2b[:], s1t, NCOLS + c0, False, t == 0)
            if t > 0:
                mm2(r2[:], s2t, c0, False, True)
            if t == 0:
                nc.vector.tensor_copy(mp[:, cs], ps2[:, 0:n])
            else:
                nc.vector.scalar_tensor_tensor(
                    mp[:, cs], mo[:, cs], BETA, ps2[:, 0:n],
                    AOp.mult, AOp.add)
            if t < T - 1:
                nc.scalar.activation(s2t[0:M2P, cs], mp[:, cs], Act.Sign,
                                     bias=negone[0:M2P, :])

        for rep in range(reps):
            warm_fill(20)
            xdma(0)
            xdma(1)
            for t in range(T):
                if t + 2 < T:
                    xdma(t + 2)
                mo, mp = m2t[t % 2], m2t[(t + 1) % 2]
                for j, (c0, n) in enumerate(sl_chunks):
                    l1(t, 0, c0, n)
                    l1(t, 1, c0, n)
                    if t < 4:
                        warm_fill(4)
                    # one paired sign over both slabs' new m1 columns
                    nc.scalar.activation(
                        s1m[:, :, c0:c0 + n], m1m[:, :, c0:c0 + n],
                        Act.Sign, bias=negone[0:M1, :])
                    # l2 group j unblocks once both slabs of its cols exist
                    l2(t, l2_chunks[j][0], l2_chunks[j][1], mo, mp)
                nc.sync.dma_start(mem_d[t], mp[:])

    if split_waits:
        _split_multi_waits(nc)
    return nc


def _v8_build_and_maps(xs, w1, w2):
    nc = build_nc_v8()
    w1h, w1l, r1, w2a, w2b, r2 = make_weights8(w1, w2)
    with ThreadPoolExecutor(8) as ex:
        xs16 = list(ex.map(lambda a: a.astype(np.float16), xs))
    ones16 = np.ones((1, NSLAB * NCOLS), np.float16)
    in_maps = [
        {"x": xs16[c], "w1h": w1h, "w1l": w1l, "r1": r1, "w2a": w2a,
         "w2b": w2b, "r2": r2, "ones16": ones16}
        for c in range(NCORES)
    ]
    return nc, in_maps


def _v7_build_and_maps(xs, w1, w2):
    nc = build_nc_v7()
    w1h, w1l, r1, w2ah, w2al, w2bh, w2bl, r2 = make_weights16(w1, w2)
    with ThreadPoolExecutor(8) as ex:
        xs16 = list(ex.map(lambda a: a.astype(np.float16), xs))
    ones16 = np.ones((1, NSLAB * NCOLS), np.float16)
    in_maps = [
        {"x": xs16[c], "w1h": w1h, "w1l": w1l, "r1": r1, "w2ah": w2ah,
         "w2al": w2al, "w2bh": w2bh, "w2bl": w2bl, "r2": r2,
         "ones16": ones16}
        for c in range(NCORES)
    ]
    return nc, in_maps


VERSION = 8


def build_for_bench(x, w1, w2):
    """Build (nc, in_maps) for external timing harnesses."""
    xpad = np.zeros((T, BPAD, NI), dtype=np.float32)
    xpad[:, :B_FULL] = np.asarray(x, np.float32)
    with ThreadPoolExecutor(8) as ex:
        xs = list(ex.map(lambda c: prep_core_x(xpad, c), range(NCORES)))
    onesv = np.ones((1, NSLAB * NCOLS), np.float32)
    if VERSION == 8:
        return _v8_build_and_maps(xs, w1, w2)
    if VERSION == 7:
        return _v7_build_and_maps(xs, w1, w2)
    if VERSION == 6:
        nc = build_nc_v6()
        w1f, r1, w2t, r2t, thr = make_weights_v6(w1, w2)
        w2a = np.ascontiguousarray(w2t[0].transpose(1, 0, 2))
        w2b = np.ascontiguousarray(w2t[1].transpose(1, 0, 2))
        r2s = np.ascontiguousarray(r2t.transpose(1, 0, 2))
        in_maps = [
            {"x": xs[c], "w1": w1f, "r1": r1, "w2a": w2a, "w2b": w2b,
             "r2": r2s, "ones": onesv}
            for c in range(NCORES)
        ]
    else:
        nc = build_nc_v5(**V5_OPTS)
        w1f, r1, w2f, r2, bI = make_weights(w1, w2)
        x16 = V5_OPTS.get("x16", False)
        if x16:
            with ThreadPoolExecutor(8) as ex:
                xs = list(ex.map(lambda a: a.astype(np.float16), xs))
            w1f = w1f.astype(np.float16)
        in_maps = [
            {"x": xs[c], "w1": w1f, "r1": r1, "w2a": w2f[0], "w2b": w2f[1],
             "r2": r2, "ones": onesv}
            for c in range(NCORES)
        ]
        if x16:
            ones16 = np.ones((1, NSLAB * NCOLS), np.float16)
            for m in in_maps:
                m["ones16"] = ones16
        if V5_OPTS.get("m2_path", "dve") != "dve":
            for m in in_maps:
                m["bI"] = bI
    return nc, in_maps


def kernel(**inputs):
    x = np.asarray(inputs["x"], dtype=np.float32)
    w1 = np.asarray(inputs["w1"], dtype=np.float32)
    w2 = np.asarray(inputs["w2"], dtype=np.float32)

    from concourse.bass_utils import run_bass_kernel_spmd

    xpad = np.zeros((T, BPAD, NI), dtype=np.float32)
    xpad[:, :B_FULL] = x
    with ThreadPoolExecutor(8) as ex:
        xs = list(ex.map(lambda c: prep_core_x(xpad, c), range(NCORES)))
    onesv = np.ones((1, NSLAB * NCOLS), np.float32)

    if VERSION == 8:
        nc, in_maps = _v8_build_and_maps(xs, w1, w2)
    elif VERSION == 7:
        nc, in_maps = _v7_build_and_maps(xs, w1, w2)
    elif VERSION == 6:
        nc = build_nc_v6()
        w1f, r1, w2t, r2t, thr = make_weights_v6(w1, w2)
        w2a = np.ascontiguousarray(w2t[0].transpose(1, 0, 2))
        w2b = np.ascontiguousarray(w2t[1].transpose(1, 0, 2))
        r2s = np.ascontiguousarray(r2t.transpose(1, 0, 2))
        in_maps = [
            {"x": xs[c], "w1": w1f, "r1": r1, "w2a": w2a, "w2b": w2b,
             "r2": r2s, "ones": onesv}
            for c in range(NCORES)
        ]
    else:
        nc = build_nc_v5(**V5_OPTS)
        w1f, r1, w2f, r2, bI = make_weights(w1, w2)
        x16 = V5_OPTS.get("x16", False)
        if x16:
            with ThreadPoolExecutor(8) as ex:
                xs = list(ex.map(lambda a: a.astype(np.float16), xs))
            w1f = w1f.astype(np.float16)
        in_maps = [
            {"x": xs[c], "w1": w1f, "r1": r1, "w2a": w2f[0], "w2b": w2f[1],
             "r2": r2, "ones": onesv}
            for c in range(NCORES)
        ]
        if x16:
            ones16 = np.ones((1, NSLAB * NCOLS), np.float16)
            for m in in_maps:
                m["ones16"] = ones16
        if V5_OPTS.get("m2_path", "dve") != "dve":
            for m in in_maps:
                m["bI"] = bI

    import time as _time
    _t0 = _time.time()
    res = run_bass_kernel_spmd(nc, in_maps, list(range(NCORES))).results
    print(f"[kernel] device compile+run {_time.time()-_t0:.1f}s", flush=True)

    if VERSION == 6:
        # vout = bf16(v - beta^-t); mem2 = beta^t*v = beta^t*vout + 1;
        # spk2 = (vout > 0) reproduces the device threshold bit-exactly.
        bt = (np.float64(BETA) ** np.arange(T)).astype(np.float32)
        spk2 = np.empty((T, BPAD, NO), dtype=np.float32)
        mem2 = np.empty((T, BPAD, NO), dtype=np.float32)

        def fill(c):
            vout = res[c]["vout"]  # [T, 126, NCOLS] bf16
            vo = np.asarray(vout, np.float32).reshape(
                T, NSLAB, NBL, NO, NCOLS).transpose(0, 1, 2, 4, 3)
            vo = np.ascontiguousarray(vo).reshape(T, BC, NO)
            sl = slice(c * BC, (c + 1) * BC)
            spk2[:, sl] = (vo > 0.0)
            mem2[:, sl] = vo * bt[:, None, None] + np.float32(1.0)

        with ThreadPoolExecutor(8) as ex:
            list(ex.map(fill, range(NCORES)))
        return spk2[:, :B_FULL], mem2[:, :B_FULL]

    mem2 = np.empty((T, BPAD, NO), dtype=np.float32)

    def fill(c):
        mem2[:, c * BC : (c + 1) * BC] = unpack_mem(res[c])

    with ThreadPoolExecutor(8) as ex:
        list(ex.map(fill, range(NCORES)))
    mem2 = mem2[:, :B_FULL]
    spk2 = (mem2 > THR).astype(np.float32)
    return spk2, mem2
